# revision 10
# baseline (speedup 1.0000x reference)
"""LGRU Bass/Tile kernel for Trainium2, 8-core data-parallel over batch.

Reference computation (per sequence step t):
    xz = x @ Wz ; xh = x @ Wh                     (input projections)
    z  = sigmoid(xz_t + h @ Uz)
    hc = relu(xh_t + h @ Uh)
    h  = z * h + (1 - z) * hc
Returns all hidden states hs[T, B, H].

Sharding: batch (B=32) split 4-per-core across 8 cores; weights replicated.

v2 design (all bf16 matmul operands -- the 2e-2 rel-err gate leaves huge
headroom; measured baseline with f32-equivalent hi/lo splitting was 4.5e-6):
  - Transposed state layout: hT[128, kc, t'*BL+b] (H on partitions) so all
    per-step elementwise work is partition-parallel.
  - Projections accumulate xzT/xhT for a whole T-block directly into PSUM;
    the per-step recurrence matmuls (h @ U, U stationary bf16) then
    accumulate ON TOP of the same PSUM columns (start=False), so the gate
    preactivations are formed entirely by the PE -- no DVE adds at all.
  - Gate chain per step: z = sigmoid(ps_z) on ACT (reads PSUM directly),
    hc = relu(ps_h) on DVE, h_new = hc + z*(h_prev - hc) (3 DVE ops),
    written straight into the bf16 hsT block buffer that the next step's
    matmuls read as their moving operand (no hT copy).
  - Half-chunk pipelining: per step the matmuls run kc{0,1} then kc{2,3}
    (mt{0,1} ordered before mt{2,3} inside the last kc); the chain for
    h-chunks {0,1} runs while the PE already starts the next step's
    kc{0,1} matmuls, hiding the serial ACT/DVE tail.
  - hsT is bf16 (output rounds to bf16: ~2e-3 rms, within budget); block
    results are PE-transposed back to natural [t*b, H] f32 and DMA'd out.
"""

import os

import numpy as np

T, B, F, H = 2048, 32, 256, 512
NCORES = 8
BL = B // NCORES  # batch per core = 4
TBLK = 64  # timesteps per block
KC = H // 128  # 4 H-chunks
FC = F // 128  # 2 F-chunks
PT = (TBLK * BL) // 128  # partition-tiles of (t,b) rows per block = 2

_CACHED = {}


def _build_nc(t_total):
    import concourse.bass as bass
    import concourse.mybir as mybir
    from concourse import bacc
    import concourse.tile as tile
    from concourse.bass import ds
    from concourse.masks import make_identity

    FD = mybir.dt.float32
    BF = mybir.dt.bfloat16
    nblk = t_total // TBLK
    NB = TBLK * BL  # block columns (t'*BL + b) = 256

    nc = bacc.Bacc("TRN2", target_bir_lowering=False, debug=False)
    x = nc.dram_tensor("x", [t_total, BL, F], FD, kind="ExternalInput")
    Wz = nc.dram_tensor("Wz", [F, H], FD, kind="ExternalInput")
    Wh = nc.dram_tensor("Wh", [F, H], FD, kind="ExternalInput")
    Uz = nc.dram_tensor("Uz", [H, H], FD, kind="ExternalInput")
    Uh = nc.dram_tensor("Uh", [H, H], FD, kind="ExternalInput")
    hs = nc.dram_tensor("hs", [t_total, BL, H], FD, kind="ExternalOutput")

    x_flat = x.rearrange("t b f -> (t b) f")  # [t_total*BL, F]
    hs_flat = hs.rearrange("t b h -> (t b) h")  # [t_total*BL, H]

    Sig = mybir.ActivationFunctionType.Sigmoid

    with tile.TileContext(nc) as tc:
        with (
            tc.tile_pool(name="const", bufs=1) as constp,
            tc.tile_pool(name="setup", bufs=2) as setupp,
            tc.tile_pool(name="state", bufs=1) as statep,
            tc.tile_pool(name="xblk", bufs=2) as xblkp,
            tc.tile_pool(name="work", bufs=3) as workp,
            tc.tile_pool(name="step", bufs=3) as stepp,
            tc.tile_pool(name="ps_gate", bufs=1, space="PSUM") as ps_gate,
            tc.tile_pool(name="ps_tr", bufs=2, space="PSUM") as ps_tr,
        ):
            ident = constp.tile([128, 128], FD, tag="ident")
            make_identity(nc, ident)
            ident_bf = constp.tile([128, 128], BF, tag="identbf")
            nc.vector.tensor_copy(ident_bf, ident)

            # --- U blocks, bf16: lhsT for (gate, kc, mt) is
            # U[g][128*kc:..., 128*mt:...] ---
            Ub = {}
            for g, Usrc in (("z", Uz), ("h", Uh)):
                for kc in range(KC):
                    stage = setupp.tile(
                        [128, H], FD, tag=f"stage{g}{kc}", name=f"stage{g}{kc}"
                    )
                    nc.sync.dma_start(out=stage, in_=Usrc[kc * 128 : (kc + 1) * 128, :])
                    ub = constp.tile([128, H], BF, tag=f"U{g}{kc}")
                    nc.vector.tensor_copy(ub, stage)
                    Ub[(g, kc)] = ub

            # --- W blocks, bf16: Wcat = [Wz | Wh] along output dim ---
            Wb = []
            for fc in range(FC):
                wtile = constp.tile([128, 2 * H], BF, tag=f"W{fc}")
                for si, Wsrc in enumerate((Wz, Wh)):
                    stage = setupp.tile(
                        [128, H], FD, tag=f"stageW{fc}{si}", name=f"stageW{fc}{si}"
                    )
                    nc.sync.dma_start(out=stage, in_=Wsrc[fc * 128 : (fc + 1) * 128, :])
                    nc.vector.tensor_copy(wtile[:, si * H : (si + 1) * H], stage)
                Wb.append(wtile)

            # --- persistent state: hsT bf16, last slice = h_{-1} = 0 ---
            hsT = statep.tile([128, KC, NB], BF)
            nc.vector.memset(hsT[:, :, (TBLK - 1) * BL :], 0.0)

            with tc.For_i(0, nblk, 1, staggered_reset=True) as blk:
                row0 = blk * NB

                # --- load x block and transpose: xT[fc] = x_blk.T chunk ---
                xT = [
                    xblkp.tile([128, NB], BF, tag=f"xT{fc}", name=f"xT{fc}")
                    for fc in range(FC)
                ]
                for pt in range(PT):
                    xin = workp.tile([128, F], FD, tag="xin", bufs=3)
                    nc.sync.dma_start(out=xin, in_=x_flat[ds(row0 + pt * 128, 128), :])
                    for fc in range(FC):
                        pst = ps_tr.tile([128, 128], FD, tag="tr")
                        nc.tensor.transpose(
                            pst, xin[:, fc * 128 : (fc + 1) * 128], ident
                        )
                        nc.vector.tensor_copy(xT[fc][:, pt * 128 : (pt + 1) * 128], pst)

                # --- projections into PSUM ---
                # Layout: one PSUM bank per mt chunk holding [xz_mt | xh_mt]
                # for the whole block. The first matmul of a group clears the
                # ENTIRE bank (hardware has_written semantics), so instead of
                # start=True we open each block with a full-bank memset and
                # run every matmul with start=False: accumulate-onto-0 equals
                # overwrite for either has_written state, and Tile's AP
                # tracking orders the memset after the previous block's reads.
                psg = ps_gate.tile([128, KC, 2, NB], FD, tag="psg")
                nc.vector.memset(psg, 0.0)
                for gi in range(2):
                    for mt in range(KC):
                        lhs_sl = slice(gi * H + mt * 128, gi * H + (mt + 1) * 128)
                        for fc in range(FC):
                            nc.tensor.matmul(
                                psg[:, mt, gi, :],
                                lhsT=Wb[fc][:, lhs_sl],
                                rhs=xT[fc],
                                start=False,
                                stop=False,
                                skip_group_check=True,
                            )

                # --- recurrence over this block ---
                for tp in range(TBLK):
                    cur = slice(tp * BL, (tp + 1) * BL)
                    prev = (
                        slice((tp - 1) * BL, tp * BL)
                        if tp > 0
                        else slice((TBLK - 1) * BL, TBLK * BL)
                    )
                    # all z-gate matmuls first, then h-gate: the sigmoid and
                    # the z-side products (f = z*h_prev, w = 1-z) compute on
                    # ACT/DVE while the PE runs the h-gate matmuls, so the
                    # exposed chain after the last h matmul is only
                    # relu -> r = w*hc -> h = f + r.
                    for gi, g in ((0, "z"), (1, "h")):
                        for kc in range(KC):
                            for mt in range(KC):
                                nc.tensor.matmul(
                                    psg[:, mt, gi, cur],
                                    lhsT=Ub[(g, kc)][:, mt * 128 : (mt + 1) * 128],
                                    rhs=hsT[:, kc, prev],
                                    start=False,
                                    stop=False,
                                    skip_group_check=True,
                                )
                    z = stepp.tile([128, KC, BL], FD, tag="z")
                    nc.scalar.activation(z, psg[:, :, 0, cur], Sig)
                    w = stepp.tile([128, KC, BL], FD, tag="w")
                    nc.vector.tensor_scalar(
                        w, z, -1.0, 1.0, mybir.AluOpType.mult, mybir.AluOpType.add
                    )
                    f = stepp.tile([128, KC, BL], FD, tag="f")
                    nc.vector.tensor_mul(f, z, hsT[:, :, prev])
                    hc = stepp.tile([128, KC, BL], FD, tag="hc")
                    nc.vector.tensor_scalar_max(hc, psg[:, :, 1, cur], 0.0)
                    r = stepp.tile([128, KC, BL], FD, tag="r")
                    nc.vector.tensor_mul(r, w, hc)
                    nc.vector.tensor_add(hsT[:, :, cur], f, r)

                # --- transpose back to natural layout and store ---
                for ct in range(PT):
                    hnat = workp.tile([128, H], FD, tag="hnat", bufs=3)
                    for c in range(KC):
                        pst = ps_tr.tile([128, 128], BF, tag="trbf")
                        nc.tensor.transpose(
                            pst, hsT[:, c, ct * 128 : (ct + 1) * 128], ident_bf
                        )
                        nc.vector.tensor_copy(hnat[:, c * 128 : (c + 1) * 128], pst)
                    nc.sync.dma_start(out=hs_flat[ds(row0 + ct * 128, 128), :], in_=hnat)

    nc.finalize()
    return nc


def kernel(x, Wz, Wh, Uz, Uh):
    from concourse.bass_utils import run_bass_kernel_spmd

    t_total = x.shape[0]
    key = (t_total,)
    if key not in _CACHED:
        _CACHED[key] = _build_nc(t_total)
    nc = _CACHED[key]

    x = np.ascontiguousarray(np.asarray(x, dtype=np.float32))
    Wz = np.ascontiguousarray(np.asarray(Wz, dtype=np.float32))
    Wh = np.ascontiguousarray(np.asarray(Wh, dtype=np.float32))
    Uz = np.ascontiguousarray(np.asarray(Uz, dtype=np.float32))
    Uh = np.ascontiguousarray(np.asarray(Uh, dtype=np.float32))

    in_maps = []
    for c in range(NCORES):
        in_maps.append(
            {
                "x": np.ascontiguousarray(x[:, c * BL : (c + 1) * BL, :]),
                "Wz": Wz,
                "Wh": Wh,
                "Uz": Uz,
                "Uh": Uh,
            }
        )

    trace = os.environ.get("LGRU_TRACE", "0") == "1"
    res = run_bass_kernel_spmd(
        nc, in_maps, core_ids=list(range(NCORES)), trace=trace
    )
    if trace and res.exec_time_ns is not None:
        print(f"HW exec time: {res.exec_time_ns} ns")
        kernel.last_exec_time_ns = res.exec_time_ns
        kernel.last_trace = res.instructions_and_trace
    out = np.concatenate([r["hs"] for r in res.results], axis=1)
    return out


# revision 13
# speedup vs baseline: 1.5093x; 1.5093x over previous
"""LGRU Bass/Tile kernel for Trainium2, 8-core data-parallel over batch.

Reference computation (per sequence step t):
    xz = x @ Wz ; xh = x @ Wh                     (input projections)
    z  = sigmoid(xz_t + h @ Uz)
    hc = relu(xh_t + h @ Uh)
    h  = z * h + (1 - z) * hc
Returns all hidden states hs[T, B, H].

Sharding: batch (B=32) split 4-per-core across 8 cores; weights replicated.

v4 design (all matmul operands bf16 -- the 2e-2 rel-err gate leaves huge
headroom; measured 3.7e-3):
  - Transposed state layout (H on partitions).  All per-step tensors use a
    [128, t-col, kc-chunk] free-dim order so one step's slab is a fully
    contiguous [128, 16] region: flat 2D APs for the ACT/DVE chain (3D
    strided APs measured ~165 ns/op vs ~70-140 flat).  The matmuls write
    4-column strided slices of the slab instead.
  - Projections accumulate xzT/xhT for a whole T-block directly into PSUM;
    per-step recurrence matmuls accumulate on top (start=False always; each
    bank is opened once per block by a DVE memset, since a start=True
    matmul clears the whole bank).
  - h is never fed to the PE as one vector: the next step's matmuls consume
    the pair [f | r] (f = z*h_prev, r = (1-z)*hc) as 8 moving columns and
    PSUM-accumulate the two products.  The exposed tail after the last
    h-gate matmul is only hc = relu(ps_h); r = w*hc -- two DVE ops.
  - w = 1-z is computed as sigmoid(-x) on ACT (scale=-1), overlapping the
    h-gate matmul phase, as do z and f.  h = f + r is materialized lazily
    during the NEXT step's burst (for the output buffer and the next f).
  - z-gate matmuls run before h-gate matmuls each step so the sigmoid path
    hides under the h-phase.
"""

import os

import numpy as np

T, B, F, H = 2048, 32, 256, 512
NCORES = 8
BL = B // NCORES  # batch per core = 4
TBLK = 64  # timesteps per block
KC = H // 128  # 4 H-chunks
FC = F // 128  # 2 F-chunks
PT = (TBLK * BL) // 128  # partition-tiles of (t,b) rows per block = 2

_CACHED = {}


def _build_nc(t_total):
    import concourse.bass as bass
    import concourse.mybir as mybir
    from concourse import bacc
    import concourse.tile as tile
    from concourse.bass import ds
    from concourse.masks import make_identity

    FD = mybir.dt.float32
    BF = mybir.dt.bfloat16
    nblk = t_total // TBLK
    NB = TBLK * BL  # block columns (t'*BL + b) = 256

    nc = bacc.Bacc("TRN2", target_bir_lowering=False, debug=False)
    x = nc.dram_tensor("x", [t_total, BL, F], FD, kind="ExternalInput")
    Wz = nc.dram_tensor("Wz", [F, H], FD, kind="ExternalInput")
    Wh = nc.dram_tensor("Wh", [F, H], FD, kind="ExternalInput")
    Uz = nc.dram_tensor("Uz", [H, H], FD, kind="ExternalInput")
    Uh = nc.dram_tensor("Uh", [H, H], FD, kind="ExternalInput")
    hs = nc.dram_tensor("hs", [t_total, BL, H], FD, kind="ExternalOutput")

    x_flat = x.rearrange("t b f -> (t b) f")
    hs_flat = hs.rearrange("t b h -> (t b) h")

    Sig = mybir.ActivationFunctionType.Sigmoid

    with tile.TileContext(nc) as tc:
        with (
            tc.tile_pool(name="const", bufs=1) as constp,
            tc.tile_pool(name="setup", bufs=2) as setupp,
            tc.tile_pool(name="state", bufs=1) as statep,
            tc.tile_pool(name="xblk", bufs=2) as xblkp,
            tc.tile_pool(name="work", bufs=3) as workp,
            tc.tile_pool(name="step", bufs=3) as stepp,
            tc.tile_pool(name="ps_gate", bufs=1, space="PSUM") as ps_gate,
            tc.tile_pool(name="ps_tr", bufs=2, space="PSUM") as ps_tr,
        ):
            ident = constp.tile([128, 128], FD, tag="ident")
            make_identity(nc, ident)
            ident_bf = constp.tile([128, 128], BF, tag="identbf")
            nc.vector.tensor_copy(ident_bf, ident)

            # --- U blocks, bf16: lhsT for (gate, kc, mt) ---
            Ub = {}
            for g, Usrc in (("z", Uz), ("h", Uh)):
                for kc in range(KC):
                    stage = setupp.tile(
                        [128, H], FD, tag=f"stage{g}{kc}", name=f"stage{g}{kc}"
                    )
                    nc.sync.dma_start(out=stage, in_=Usrc[kc * 128 : (kc + 1) * 128, :])
                    ub = constp.tile([128, H], BF, tag=f"U{g}{kc}")
                    nc.vector.tensor_copy(ub, stage)
                    Ub[(g, kc)] = ub

            # --- W blocks, bf16: Wcat = [Wz | Wh] along output dim ---
            Wb = []
            for fc in range(FC):
                wtile = constp.tile([128, 2 * H], BF, tag=f"W{fc}")
                for si, Wsrc in enumerate((Wz, Wh)):
                    stage = setupp.tile(
                        [128, H], FD, tag=f"stageW{fc}{si}", name=f"stageW{fc}{si}"
                    )
                    nc.sync.dma_start(out=stage, in_=Wsrc[fc * 128 : (fc + 1) * 128, :])
                    nc.vector.tensor_copy(wtile[:, si * H : (si + 1) * H], stage)
                Wb.append(wtile)

            # --- persistent state ---
            # hsT[128, col=t'*BL+b, kc] bf16: h states of the current block.
            hsT = statep.tile([128, NB, KC], BF)
            nc.vector.memset(hsT[:, (TBLK - 1) * BL :, :], 0.0)
            # fr[128, slot, {f,r}, b, kc] bf16: the split-h moving operand,
            # double-slotted by step parity.  Slot 1 = state before step 0.
            fr = statep.tile([128, 2, 2, BL, KC], BF)
            nc.vector.memset(fr[:, 1, :, :, :], 0.0)

            with tc.For_i(0, nblk, 1, staggered_reset=True) as blk:
                row0 = blk * NB

                # --- load x block and transpose: xT[fc] = x_blk.T chunk ---
                xT = [
                    xblkp.tile([128, NB], BF, tag=f"xT{fc}", name=f"xT{fc}")
                    for fc in range(FC)
                ]
                for pt in range(PT):
                    xin = workp.tile([128, F], FD, tag="xin", bufs=3)
                    nc.sync.dma_start(out=xin, in_=x_flat[ds(row0 + pt * 128, 128), :])
                    for fc in range(FC):
                        pst = ps_tr.tile([128, 128], FD, tag="tr")
                        nc.tensor.transpose(
                            pst, xin[:, fc * 128 : (fc + 1) * 128], ident
                        )
                        nc.vector.tensor_copy(xT[fc][:, pt * 128 : (pt + 1) * 128], pst)

                # --- projections into PSUM ---
                # psg_*[128, col, mt]: step slab [:, cur, :] is contiguous.
                # Full-tile memset opens the banks (no start=True anywhere:
                # a start matmul would clear whole banks).
                psg_z = ps_gate.tile([128, NB, KC], FD, tag="psz")
                psg_h = ps_gate.tile([128, NB, KC], FD, tag="psh")
                nc.vector.memset(psg_z, 0.0)
                nc.vector.memset(psg_h, 0.0)
                for gi, psg in ((0, psg_z), (1, psg_h)):
                    for mt in range(KC):
                        lhs_sl = slice(gi * H + mt * 128, gi * H + (mt + 1) * 128)
                        for fc in range(FC):
                            nc.tensor.matmul(
                                psg[:, :, mt],
                                lhsT=Wb[fc][:, lhs_sl],
                                rhs=xT[fc],
                                start=False,
                                stop=False,
                                skip_group_check=True,
                            )

                # --- recurrence over this block ---
                for tp in range(TBLK):
                    cur = slice(tp * BL, (tp + 1) * BL)
                    prev = (
                        slice((tp - 1) * BL, tp * BL)
                        if tp > 0
                        else slice((TBLK - 1) * BL, TBLK * BL)
                    )
                    s = tp % 2
                    sp = 1 - s  # slot holding [f|r] of step tp-1
                    # z-gate matmuls first, then h-gate: the sigmoid path
                    # (z, w, f) hides under the h-phase.
                    # The out AP aliases its 4 columns twice (zero-stride
                    # leading free dim) so the f- and r-products of the 8
                    # moving columns accumulate into the same PSUM columns.
                    for gi, g, psg in ((0, "z", psg_z), (1, "h", psg_h)):
                        for kc in range(KC):
                            for mt in range(KC):
                                out = psg[:, cur, mt]
                                out2 = out.copy()
                                out2.ap = out2.ap[:1] + [[0, 2]] + out2.ap[1:]
                                nc.tensor.matmul(
                                    out2,
                                    lhsT=Ub[(g, kc)][:, mt * 128 : (mt + 1) * 128],
                                    rhs=fr[:, sp, :, :, kc],
                                    start=False,
                                    stop=False,
                                    skip_group_check=True,
                                )
                    # h_{t-1} = f+r materialized lazily (output + next f)
                    if tp > 0:
                        nc.vector.tensor_add(
                            hsT[:, prev, :], fr[:, sp, 0, :, :], fr[:, sp, 1, :, :]
                        )
                    z = stepp.tile([128, BL, KC], FD, tag="z")
                    nc.scalar.activation(z, psg_z[:, cur, :], Sig)
                    w = stepp.tile([128, BL, KC], FD, tag="w")
                    nc.scalar.activation(w, psg_z[:, cur, :], Sig, scale=-1.0)
                    nc.vector.tensor_mul(fr[:, s, 0, :, :], z, hsT[:, prev, :])
                    # exposed tail after the last h-gate matmul:
                    hc = stepp.tile([128, BL, KC], FD, tag="hc")
                    nc.vector.tensor_scalar_max(hc, psg_h[:, cur, :], 0.0)
                    nc.vector.tensor_mul(fr[:, s, 1, :, :], w, hc)

                # last step's h for this block
                lastsl = slice((TBLK - 1) * BL, TBLK * BL)
                lsp = (TBLK - 1) % 2
                nc.vector.tensor_add(
                    hsT[:, lastsl, :], fr[:, lsp, 0, :, :], fr[:, lsp, 1, :, :]
                )

                # --- transpose back to natural layout and store ---
                for ct in range(PT):
                    hnat = workp.tile([128, H], FD, tag="hnat", bufs=3)
                    for c in range(KC):
                        pst = ps_tr.tile([128, 128], BF, tag="trbf")
                        nc.tensor.transpose(
                            pst, hsT[:, ct * 128 : (ct + 1) * 128, c], ident_bf
                        )
                        nc.vector.tensor_copy(hnat[:, c * 128 : (c + 1) * 128], pst)
                    nc.sync.dma_start(out=hs_flat[ds(row0 + ct * 128, 128), :], in_=hnat)

    nc.finalize()
    return nc


def kernel(x, Wz, Wh, Uz, Uh):
    from concourse.bass_utils import run_bass_kernel_spmd

    t_total = x.shape[0]
    key = (t_total,)
    if key not in _CACHED:
        _CACHED[key] = _build_nc(t_total)
    nc = _CACHED[key]

    x = np.ascontiguousarray(np.asarray(x, dtype=np.float32))
    Wz = np.ascontiguousarray(np.asarray(Wz, dtype=np.float32))
    Wh = np.ascontiguousarray(np.asarray(Wh, dtype=np.float32))
    Uz = np.ascontiguousarray(np.asarray(Uz, dtype=np.float32))
    Uh = np.ascontiguousarray(np.asarray(Uh, dtype=np.float32))

    in_maps = []
    for c in range(NCORES):
        in_maps.append(
            {
                "x": np.ascontiguousarray(x[:, c * BL : (c + 1) * BL, :]),
                "Wz": Wz,
                "Wh": Wh,
                "Uz": Uz,
                "Uh": Uh,
            }
        )

    trace = os.environ.get("LGRU_TRACE", "0") == "1"
    res = run_bass_kernel_spmd(
        nc, in_maps, core_ids=list(range(NCORES)), trace=trace
    )
    if trace and res.exec_time_ns is not None:
        print(f"HW exec time: {res.exec_time_ns} ns")
        kernel.last_exec_time_ns = res.exec_time_ns
        kernel.last_trace = res.instructions_and_trace
    out = np.concatenate([r["hs"] for r in res.results], axis=1)
    return out


# revision 14
# speedup vs baseline: 1.5116x; 1.0015x over previous
"""LGRU Bass/Tile kernel for Trainium2, 8-core data-parallel over batch.

Reference computation (per sequence step t):
    xz = x @ Wz ; xh = x @ Wh                     (input projections)
    z  = sigmoid(xz_t + h @ Uz)
    hc = relu(xh_t + h @ Uh)
    h  = z * h + (1 - z) * hc
Returns all hidden states hs[T, B, H].

Sharding: batch (B=32) split 4-per-core across 8 cores; weights replicated.

v4 design (all matmul operands bf16 -- the 2e-2 rel-err gate leaves huge
headroom; measured 3.7e-3):
  - Transposed state layout (H on partitions).  All per-step tensors use a
    [128, t-col, kc-chunk] free-dim order so one step's slab is a fully
    contiguous [128, 16] region: flat 2D APs for the ACT/DVE chain (3D
    strided APs measured ~165 ns/op vs ~70-140 flat).  The matmuls write
    4-column strided slices of the slab instead.
  - Projections accumulate xzT/xhT for a whole T-block directly into PSUM;
    per-step recurrence matmuls accumulate on top (start=False always; each
    bank is opened once per block by a DVE memset, since a start=True
    matmul clears the whole bank).
  - h is never fed to the PE as one vector: the next step's matmuls consume
    the pair [f | r] (f = z*h_prev, r = (1-z)*hc) as 8 moving columns and
    PSUM-accumulate the two products.  The exposed tail after the last
    h-gate matmul is only hc = relu(ps_h); r = w*hc -- two DVE ops.
  - w = 1-z is computed as sigmoid(-x) on ACT (scale=-1), overlapping the
    h-gate matmul phase, as do z and f.  h = f + r is materialized lazily
    during the NEXT step's burst (for the output buffer and the next f).
  - z-gate matmuls run before h-gate matmuls each step so the sigmoid path
    hides under the h-phase.
"""

import os

import numpy as np

T, B, F, H = 2048, 32, 256, 512
NCORES = 8
BL = B // NCORES  # batch per core = 4
TBLK = 64  # timesteps per block
KC = H // 128  # 4 H-chunks
FC = F // 128  # 2 F-chunks
PT = (TBLK * BL) // 128  # partition-tiles of (t,b) rows per block = 2

_CACHED = {}


def _build_nc(t_total):
    import concourse.bass as bass
    import concourse.mybir as mybir
    from concourse import bacc
    import concourse.tile as tile
    from concourse.bass import ds
    from concourse.masks import make_identity

    FD = mybir.dt.float32
    BF = mybir.dt.bfloat16
    nblk = t_total // TBLK
    NB = TBLK * BL  # block columns (t'*BL + b) = 256

    nc = bacc.Bacc("TRN2", target_bir_lowering=False, debug=False)
    x = nc.dram_tensor("x", [t_total, BL, F], FD, kind="ExternalInput")
    Wz = nc.dram_tensor("Wz", [F, H], FD, kind="ExternalInput")
    Wh = nc.dram_tensor("Wh", [F, H], FD, kind="ExternalInput")
    Uz = nc.dram_tensor("Uz", [H, H], FD, kind="ExternalInput")
    Uh = nc.dram_tensor("Uh", [H, H], FD, kind="ExternalInput")
    hs = nc.dram_tensor("hs", [t_total, BL, H], FD, kind="ExternalOutput")

    x_flat = x.rearrange("t b f -> (t b) f")
    hs_flat = hs.rearrange("t b h -> (t b) h")

    Sig = mybir.ActivationFunctionType.Sigmoid

    with tile.TileContext(nc) as tc:
        with (
            tc.tile_pool(name="const", bufs=1) as constp,
            tc.tile_pool(name="setup", bufs=2) as setupp,
            tc.tile_pool(name="state", bufs=1) as statep,
            tc.tile_pool(name="xblk", bufs=2) as xblkp,
            tc.tile_pool(name="work", bufs=3) as workp,
            tc.tile_pool(name="step", bufs=3) as stepp,
            tc.tile_pool(name="ps_gate", bufs=1, space="PSUM") as ps_gate,
            tc.tile_pool(name="ps_tr", bufs=2, space="PSUM") as ps_tr,
        ):
            ident = constp.tile([128, 128], FD, tag="ident")
            make_identity(nc, ident)
            ident_bf = constp.tile([128, 128], BF, tag="identbf")
            nc.vector.tensor_copy(ident_bf, ident)

            # --- U blocks, bf16: lhsT for (gate, kc, mt) ---
            Ub = {}
            for g, Usrc in (("z", Uz), ("h", Uh)):
                for kc in range(KC):
                    stage = setupp.tile(
                        [128, H], FD, tag=f"stage{g}{kc}", name=f"stage{g}{kc}"
                    )
                    nc.sync.dma_start(out=stage, in_=Usrc[kc * 128 : (kc + 1) * 128, :])
                    ub = constp.tile([128, H], BF, tag=f"U{g}{kc}")
                    nc.vector.tensor_copy(ub, stage)
                    Ub[(g, kc)] = ub

            # --- W blocks, bf16: Wcat = [Wz | Wh] along output dim ---
            Wb = []
            for fc in range(FC):
                wtile = constp.tile([128, 2 * H], BF, tag=f"W{fc}")
                for si, Wsrc in enumerate((Wz, Wh)):
                    stage = setupp.tile(
                        [128, H], FD, tag=f"stageW{fc}{si}", name=f"stageW{fc}{si}"
                    )
                    nc.sync.dma_start(out=stage, in_=Wsrc[fc * 128 : (fc + 1) * 128, :])
                    nc.vector.tensor_copy(wtile[:, si * H : (si + 1) * H], stage)
                Wb.append(wtile)

            # --- persistent state ---
            # hsT[128, col=t'*BL+b, kc] bf16: h states of the current block.
            hsT = statep.tile([128, NB, KC], BF)
            nc.vector.memset(hsT[:, (TBLK - 1) * BL :, :], 0.0)
            # fr[128, slot, {f,r}, b, kc] bf16: the split-h moving operand,
            # double-slotted by step parity.  Slot 1 = state before step 0.
            fr = statep.tile([128, 2, 2, BL, KC], BF)
            nc.vector.memset(fr[:, 1, :, :, :], 0.0)

            with tc.For_i(0, nblk, 1, staggered_reset=True) as blk:
                row0 = blk * NB

                # --- load x block and transpose: xT[fc] = x_blk.T chunk ---
                xT = [
                    xblkp.tile([128, NB], BF, tag=f"xT{fc}", name=f"xT{fc}")
                    for fc in range(FC)
                ]
                for pt in range(PT):
                    xin = workp.tile([128, F], FD, tag="xin", bufs=3)
                    nc.sync.dma_start(out=xin, in_=x_flat[ds(row0 + pt * 128, 128), :])
                    for fc in range(FC):
                        pst = ps_tr.tile([128, 128], FD, tag="tr")
                        nc.tensor.transpose(
                            pst, xin[:, fc * 128 : (fc + 1) * 128], ident
                        )
                        nc.vector.tensor_copy(xT[fc][:, pt * 128 : (pt + 1) * 128], pst)

                # --- projections into PSUM ---
                # psg_*[128, col, mt]: step slab [:, cur, :] is contiguous.
                # Full-tile memset opens the banks (no start=True anywhere:
                # a start matmul would clear whole banks).
                psg_z = ps_gate.tile([128, NB, KC], FD, tag="psz")
                psg_h = ps_gate.tile([128, NB, KC], FD, tag="psh")
                nc.vector.memset(psg_z, 0.0)
                nc.vector.memset(psg_h, 0.0)
                for gi, psg in ((0, psg_z), (1, psg_h)):
                    for mt in range(KC):
                        lhs_sl = slice(gi * H + mt * 128, gi * H + (mt + 1) * 128)
                        for fc in range(FC):
                            nc.tensor.matmul(
                                psg[:, :, mt],
                                lhsT=Wb[fc][:, lhs_sl],
                                rhs=xT[fc],
                                start=False,
                                stop=False,
                                skip_group_check=True,
                            )

                # --- recurrence over this block ---
                for tp in range(TBLK):
                    cur = slice(tp * BL, (tp + 1) * BL)
                    prev = (
                        slice((tp - 1) * BL, tp * BL)
                        if tp > 0
                        else slice((TBLK - 1) * BL, TBLK * BL)
                    )
                    s = tp % 2
                    sp = 1 - s  # slot holding [f|r] of step tp-1
                    # z-gate matmuls first, then h-gate: the sigmoid path
                    # (z, w, f) hides under the h-phase.
                    # The out AP aliases its 4 columns twice (zero-stride
                    # leading free dim) so the f- and r-products of the 8
                    # moving columns accumulate into the same PSUM columns.
                    for gi, g, psg in ((0, "z", psg_z), (1, "h", psg_h)):
                        for kc in range(KC):
                            for mt in range(KC):
                                out = psg[:, cur, mt]
                                out2 = out.copy()
                                out2.ap = out2.ap[:1] + [[0, 2]] + out2.ap[1:]
                                nc.tensor.matmul(
                                    out2,
                                    lhsT=Ub[(g, kc)][:, mt * 128 : (mt + 1) * 128],
                                    rhs=fr[:, sp, :, :, kc],
                                    start=False,
                                    stop=False,
                                    skip_group_check=True,
                                )
                    # h_{t-1} = f+r materialized lazily on GPSIMD (used by the
                    # output buffer and the next step's f; off critical path)
                    if tp > 0:
                        nc.gpsimd.tensor_add(
                            hsT[:, prev, :], fr[:, sp, 0, :, :], fr[:, sp, 1, :, :]
                        )
                    z = stepp.tile([128, BL, KC], FD, tag="z")
                    nc.scalar.activation(z, psg_z[:, cur, :], Sig)
                    # w = 1-z on DVE (an ACT op would serialize behind the
                    # sigmoid and add a second ACT->DVE semaphore hop)
                    w = stepp.tile([128, BL, KC], FD, tag="w")
                    nc.vector.tensor_scalar(
                        w, z, -1.0, 1.0, mybir.AluOpType.mult, mybir.AluOpType.add
                    )
                    # f = z * h_prev on GPSIMD, parallel to DVE
                    nc.gpsimd.tensor_mul(fr[:, s, 0, :, :], z, hsT[:, prev, :])
                    # exposed tail after the last h-gate matmul: ONE fused op
                    # r = (ps_h max 0) * w
                    nc.vector.scalar_tensor_tensor(
                        fr[:, s, 1, :, :],
                        psg_h[:, cur, :],
                        0.0,
                        w,
                        mybir.AluOpType.max,
                        mybir.AluOpType.mult,
                    )

                # last step's h for this block
                lastsl = slice((TBLK - 1) * BL, TBLK * BL)
                lsp = (TBLK - 1) % 2
                nc.gpsimd.tensor_add(
                    hsT[:, lastsl, :], fr[:, lsp, 0, :, :], fr[:, lsp, 1, :, :]
                )

                # --- transpose back to natural layout and store ---
                for ct in range(PT):
                    hnat = workp.tile([128, H], FD, tag="hnat", bufs=3)
                    for c in range(KC):
                        pst = ps_tr.tile([128, 128], BF, tag="trbf")
                        nc.tensor.transpose(
                            pst, hsT[:, ct * 128 : (ct + 1) * 128, c], ident_bf
                        )
                        nc.vector.tensor_copy(hnat[:, c * 128 : (c + 1) * 128], pst)
                    nc.sync.dma_start(out=hs_flat[ds(row0 + ct * 128, 128), :], in_=hnat)

    nc.finalize()
    return nc


def kernel(x, Wz, Wh, Uz, Uh):
    from concourse.bass_utils import run_bass_kernel_spmd

    t_total = x.shape[0]
    key = (t_total,)
    if key not in _CACHED:
        _CACHED[key] = _build_nc(t_total)
    nc = _CACHED[key]

    x = np.ascontiguousarray(np.asarray(x, dtype=np.float32))
    Wz = np.ascontiguousarray(np.asarray(Wz, dtype=np.float32))
    Wh = np.ascontiguousarray(np.asarray(Wh, dtype=np.float32))
    Uz = np.ascontiguousarray(np.asarray(Uz, dtype=np.float32))
    Uh = np.ascontiguousarray(np.asarray(Uh, dtype=np.float32))

    in_maps = []
    for c in range(NCORES):
        in_maps.append(
            {
                "x": np.ascontiguousarray(x[:, c * BL : (c + 1) * BL, :]),
                "Wz": Wz,
                "Wh": Wh,
                "Uz": Uz,
                "Uh": Uh,
            }
        )

    trace = os.environ.get("LGRU_TRACE", "0") == "1"
    res = run_bass_kernel_spmd(
        nc, in_maps, core_ids=list(range(NCORES)), trace=trace
    )
    if trace and res.exec_time_ns is not None:
        print(f"HW exec time: {res.exec_time_ns} ns")
        kernel.last_exec_time_ns = res.exec_time_ns
        kernel.last_trace = res.instructions_and_trace
    out = np.concatenate([r["hs"] for r in res.results], axis=1)
    return out


# revision 16
# speedup vs baseline: 1.5153x; 1.0025x over previous
"""LGRU Bass/Tile kernel for Trainium2, 8-core data-parallel over batch.

Reference computation (per sequence step t):
    xz = x @ Wz ; xh = x @ Wh                     (input projections)
    z  = sigmoid(xz_t + h @ Uz)
    hc = relu(xh_t + h @ Uh)
    h  = z * h + (1 - z) * hc
Returns all hidden states hs[T, B, H].

Sharding: batch (B=32) split 4-per-core across 8 cores; weights replicated.

v4 design (all matmul operands bf16 -- the 2e-2 rel-err gate leaves huge
headroom; measured 3.7e-3):
  - Transposed state layout (H on partitions).  All per-step tensors use a
    [128, t-col, kc-chunk] free-dim order so one step's slab is a fully
    contiguous [128, 16] region: flat 2D APs for the ACT/DVE chain (3D
    strided APs measured ~165 ns/op vs ~70-140 flat).  The matmuls write
    4-column strided slices of the slab instead.
  - Projections accumulate xzT/xhT for a whole T-block directly into PSUM;
    per-step recurrence matmuls accumulate on top (start=False always; each
    bank is opened once per block by a DVE memset, since a start=True
    matmul clears the whole bank).
  - h is never fed to the PE as one vector: the next step's matmuls consume
    the pair [f | r] (f = z*h_prev, r = (1-z)*hc) as 8 moving columns and
    PSUM-accumulate the two products.  The exposed tail after the last
    h-gate matmul is only hc = relu(ps_h); r = w*hc -- two DVE ops.
  - w = 1-z is computed as sigmoid(-x) on ACT (scale=-1), overlapping the
    h-gate matmul phase, as do z and f.  h = f + r is materialized lazily
    during the NEXT step's burst (for the output buffer and the next f).
  - z-gate matmuls run before h-gate matmuls each step so the sigmoid path
    hides under the h-phase.
"""

import os

import numpy as np

T, B, F, H = 2048, 32, 256, 512
NCORES = 8
BL = B // NCORES  # batch per core = 4
TBLK = 64  # timesteps per block
KC = H // 128  # 4 H-chunks
FC = F // 128  # 2 F-chunks
PT = (TBLK * BL) // 128  # partition-tiles of (t,b) rows per block = 2

_CACHED = {}


def _build_nc(t_total):
    import concourse.bass as bass
    import concourse.mybir as mybir
    from concourse import bacc
    import concourse.tile as tile
    from concourse.bass import ds
    from concourse.masks import make_identity

    FD = mybir.dt.float32
    BF = mybir.dt.bfloat16
    nblk = t_total // TBLK
    NB = TBLK * BL  # block columns (t'*BL + b) = 256

    nc = bacc.Bacc("TRN2", target_bir_lowering=False, debug=False)
    x = nc.dram_tensor("x", [t_total, BL, F], FD, kind="ExternalInput")
    Wz = nc.dram_tensor("Wz", [F, H], FD, kind="ExternalInput")
    Wh = nc.dram_tensor("Wh", [F, H], FD, kind="ExternalInput")
    Uz = nc.dram_tensor("Uz", [H, H], FD, kind="ExternalInput")
    Uh = nc.dram_tensor("Uh", [H, H], FD, kind="ExternalInput")
    hs = nc.dram_tensor("hs", [t_total, BL, H], FD, kind="ExternalOutput")

    x_flat = x.rearrange("t b f -> (t b) f")
    hs_flat = hs.rearrange("t b h -> (t b) h")

    Sig = mybir.ActivationFunctionType.Sigmoid

    with tile.TileContext(nc) as tc:
        with (
            tc.tile_pool(name="const", bufs=1) as constp,
            tc.tile_pool(name="setup", bufs=2) as setupp,
            tc.tile_pool(name="state", bufs=1) as statep,
            tc.tile_pool(name="xblk", bufs=2) as xblkp,
            tc.tile_pool(name="work", bufs=3) as workp,
            tc.tile_pool(name="step", bufs=3) as stepp,
            tc.tile_pool(name="ps_gate", bufs=1, space="PSUM") as ps_gate,
            tc.tile_pool(name="ps_tr", bufs=2, space="PSUM") as ps_tr,
        ):
            ident = constp.tile([128, 128], FD, tag="ident")
            make_identity(nc, ident)
            ident_bf = constp.tile([128, 128], BF, tag="identbf")
            nc.vector.tensor_copy(ident_bf, ident)

            # --- U blocks, bf16: lhsT for (gate, kc, mt) ---
            Ub = {}
            for g, Usrc in (("z", Uz), ("h", Uh)):
                for kc in range(KC):
                    stage = setupp.tile(
                        [128, H], FD, tag=f"stage{g}{kc}", name=f"stage{g}{kc}"
                    )
                    nc.sync.dma_start(out=stage, in_=Usrc[kc * 128 : (kc + 1) * 128, :])
                    ub = constp.tile([128, H], BF, tag=f"U{g}{kc}")
                    nc.vector.tensor_copy(ub, stage)
                    Ub[(g, kc)] = ub

            # --- W blocks, bf16: Wcat = [Wz | Wh] along output dim ---
            Wb = []
            for fc in range(FC):
                wtile = constp.tile([128, 2 * H], BF, tag=f"W{fc}")
                for si, Wsrc in enumerate((Wz, Wh)):
                    stage = setupp.tile(
                        [128, H], FD, tag=f"stageW{fc}{si}", name=f"stageW{fc}{si}"
                    )
                    nc.sync.dma_start(out=stage, in_=Wsrc[fc * 128 : (fc + 1) * 128, :])
                    nc.vector.tensor_copy(wtile[:, si * H : (si + 1) * H], stage)
                Wb.append(wtile)

            # --- persistent state ---
            # hsT[128, col=t'*BL+b, kc] bf16: h states of the current block.
            hsT = statep.tile([128, NB, KC], BF)
            nc.vector.memset(hsT[:, (TBLK - 1) * BL :, :], 0.0)
            # fr[128, slot, {f,r}, b, kc] bf16: the split-h moving operand,
            # double-slotted by step parity.  Slot 1 = state before step 0.
            fr = statep.tile([128, 2, 2, BL, KC], BF)
            nc.vector.memset(fr[:, 1, :, :, :], 0.0)

            with tc.For_i(0, nblk, 1, staggered_reset=True) as blk:
                row0 = blk * NB

                # --- load x block and transpose: xT[fc] = x_blk.T chunk ---
                xT = [
                    xblkp.tile([128, NB], BF, tag=f"xT{fc}", name=f"xT{fc}")
                    for fc in range(FC)
                ]
                for pt in range(PT):
                    xin = workp.tile([128, F], FD, tag="xin", bufs=3)
                    nc.sync.dma_start(out=xin, in_=x_flat[ds(row0 + pt * 128, 128), :])
                    for fc in range(FC):
                        pst = ps_tr.tile([128, 128], FD, tag="tr")
                        nc.tensor.transpose(
                            pst, xin[:, fc * 128 : (fc + 1) * 128], ident
                        )
                        nc.vector.tensor_copy(xT[fc][:, pt * 128 : (pt + 1) * 128], pst)

                # --- projections into PSUM ---
                # psg_*[128, col, mt]: step slab [:, cur, :] is contiguous.
                # Full-tile memset opens the banks each block (no start=True
                # anywhere: a start matmul clears the whole PSUM bank, which
                # is unsafe with multiple groups per bank; accumulate-onto-0
                # equals overwrite for either has_written state).
                psg_z = ps_gate.tile([128, NB, KC], FD, tag="psz")
                psg_h = ps_gate.tile([128, NB, KC], FD, tag="psh")
                nc.vector.memset(psg_z, 0.0)
                nc.vector.memset(psg_h, 0.0)
                for gi, psg in ((0, psg_z), (1, psg_h)):
                    for mt in range(KC):
                        lhs_sl = slice(gi * H + mt * 128, gi * H + (mt + 1) * 128)
                        for fc in range(FC):
                            nc.tensor.matmul(
                                psg[:, :, mt],
                                lhsT=Wb[fc][:, lhs_sl],
                                rhs=xT[fc],
                                start=False,
                                stop=False,
                                skip_group_check=True,
                            )

                # --- recurrence over this block ---
                for tp in range(TBLK):
                    cur = slice(tp * BL, (tp + 1) * BL)
                    prev = (
                        slice((tp - 1) * BL, tp * BL)
                        if tp > 0
                        else slice((TBLK - 1) * BL, TBLK * BL)
                    )
                    s = tp % 2
                    sp = 1 - s  # slot holding [f|r] of step tp-1
                    # z-gate matmuls first, then h-gate: the sigmoid path
                    # (z, w, f) hides under the h-phase.
                    # The out AP aliases its 4 columns twice (zero-stride
                    # leading free dim) so the f- and r-products of the 8
                    # moving columns accumulate into the same PSUM columns.
                    for gi, g, psg in ((0, "z", psg_z), (1, "h", psg_h)):
                        for kc in range(KC):
                            for mt in range(KC):
                                out = psg[:, cur, mt]
                                out2 = out.copy()
                                out2.ap = out2.ap[:1] + [[0, 2]] + out2.ap[1:]
                                nc.tensor.matmul(
                                    out2,
                                    lhsT=Ub[(g, kc)][:, mt * 128 : (mt + 1) * 128],
                                    rhs=fr[:, sp, :, :, kc],
                                    start=False,
                                    stop=False,
                                    skip_group_check=True,
                                )
                    # h_{t-1} = f+r materialized lazily on GPSIMD (used by the
                    # output buffer and the next step's f; off critical path)
                    if tp > 0:
                        nc.gpsimd.tensor_add(
                            hsT[:, prev, :], fr[:, sp, 0, :, :], fr[:, sp, 1, :, :]
                        )
                    z = stepp.tile([128, BL, KC], FD, tag="z")
                    nc.scalar.activation(z, psg_z[:, cur, :], Sig)
                    # w = 1-z on DVE (an ACT op would serialize behind the
                    # sigmoid and add a second ACT->DVE semaphore hop)
                    w = stepp.tile([128, BL, KC], FD, tag="w")
                    nc.vector.tensor_scalar(
                        w, z, -1.0, 1.0, mybir.AluOpType.mult, mybir.AluOpType.add
                    )
                    # f = z * h_prev on GPSIMD, parallel to DVE
                    nc.gpsimd.tensor_mul(fr[:, s, 0, :, :], z, hsT[:, prev, :])
                    # exposed tail after the last h-gate matmul: ONE fused op
                    # r = (ps_h max 0) * w
                    nc.vector.scalar_tensor_tensor(
                        fr[:, s, 1, :, :],
                        psg_h[:, cur, :],
                        0.0,
                        w,
                        mybir.AluOpType.max,
                        mybir.AluOpType.mult,
                    )

                # last step's h for this block
                lastsl = slice((TBLK - 1) * BL, TBLK * BL)
                lsp = (TBLK - 1) % 2
                nc.gpsimd.tensor_add(
                    hsT[:, lastsl, :], fr[:, lsp, 0, :, :], fr[:, lsp, 1, :, :]
                )

                # --- transpose back to natural layout and store ---
                for ct in range(PT):
                    hnat = workp.tile([128, H], FD, tag="hnat", bufs=3)
                    for c in range(KC):
                        pst = ps_tr.tile([128, 128], BF, tag="trbf")
                        nc.tensor.transpose(
                            pst, hsT[:, ct * 128 : (ct + 1) * 128, c], ident_bf
                        )
                        nc.vector.tensor_copy(hnat[:, c * 128 : (c + 1) * 128], pst)
                    nc.sync.dma_start(out=hs_flat[ds(row0 + ct * 128, 128), :], in_=hnat)

    nc.finalize()
    return nc


def kernel(x, Wz, Wh, Uz, Uh):
    from concourse.bass_utils import run_bass_kernel_spmd

    t_total = x.shape[0]
    key = (t_total,)
    if key not in _CACHED:
        _CACHED[key] = _build_nc(t_total)
    nc = _CACHED[key]

    x = np.ascontiguousarray(np.asarray(x, dtype=np.float32))
    Wz = np.ascontiguousarray(np.asarray(Wz, dtype=np.float32))
    Wh = np.ascontiguousarray(np.asarray(Wh, dtype=np.float32))
    Uz = np.ascontiguousarray(np.asarray(Uz, dtype=np.float32))
    Uh = np.ascontiguousarray(np.asarray(Uh, dtype=np.float32))

    in_maps = []
    for c in range(NCORES):
        in_maps.append(
            {
                "x": np.ascontiguousarray(x[:, c * BL : (c + 1) * BL, :]),
                "Wz": Wz,
                "Wh": Wh,
                "Uz": Uz,
                "Uh": Uh,
            }
        )

    trace = os.environ.get("LGRU_TRACE", "0") == "1"
    res = run_bass_kernel_spmd(
        nc, in_maps, core_ids=list(range(NCORES)), trace=trace
    )
    if trace and res.exec_time_ns is not None:
        print(f"HW exec time: {res.exec_time_ns} ns")
        kernel.last_exec_time_ns = res.exec_time_ns
        kernel.last_trace = res.instructions_and_trace
    out = np.concatenate([r["hs"] for r in res.results], axis=1)
    return out


# revision 19
# speedup vs baseline: 1.5334x; 1.0120x over previous
"""LGRU Bass/Tile kernel for Trainium2, 8-core data-parallel over batch.

Reference computation (per sequence step t):
    xz = x @ Wz ; xh = x @ Wh                     (input projections)
    z  = sigmoid(xz_t + h @ Uz)
    hc = relu(xh_t + h @ Uh)
    h  = z * h + (1 - z) * hc
Returns all hidden states hs[T, B, H].

Sharding: batch (B=32) split 4-per-core across 8 cores; weights replicated.

v4 design (all matmul operands bf16 -- the 2e-2 rel-err gate leaves huge
headroom; measured 3.7e-3):
  - Transposed state layout (H on partitions).  All per-step tensors use a
    [128, t-col, kc-chunk] free-dim order so one step's slab is a fully
    contiguous [128, 16] region: flat 2D APs for the ACT/DVE chain (3D
    strided APs measured ~165 ns/op vs ~70-140 flat).  The matmuls write
    4-column strided slices of the slab instead.
  - Projections accumulate xzT/xhT for a whole T-block directly into PSUM;
    per-step recurrence matmuls accumulate on top (start=False always; each
    bank is opened once per block by a DVE memset, since a start=True
    matmul clears the whole bank).
  - h is never fed to the PE as one vector: the next step's matmuls consume
    the pair [f | r] (f = z*h_prev, r = (1-z)*hc) as 8 moving columns and
    PSUM-accumulate the two products.  The exposed tail after the last
    h-gate matmul is only hc = relu(ps_h); r = w*hc -- two DVE ops.
  - w = 1-z is computed as sigmoid(-x) on ACT (scale=-1), overlapping the
    h-gate matmul phase, as do z and f.  h = f + r is materialized lazily
    during the NEXT step's burst (for the output buffer and the next f).
  - z-gate matmuls run before h-gate matmuls each step so the sigmoid path
    hides under the h-phase.
"""

import os

import numpy as np

T, B, F, H = 2048, 32, 256, 512
NCORES = 8
BL = B // NCORES  # batch per core = 4
TBLK = 64  # timesteps per block
KC = H // 128  # 4 H-chunks
FC = F // 128  # 2 F-chunks
PT = (TBLK * BL) // 128  # partition-tiles of (t,b) rows per block = 2

_CACHED = {}


def _build_nc(t_total):
    import concourse.bass as bass
    import concourse.mybir as mybir
    from concourse import bacc
    import concourse.tile as tile
    from concourse.bass import ds
    from concourse.masks import make_identity

    FD = mybir.dt.float32
    BF = mybir.dt.bfloat16
    nblk = t_total // TBLK
    NB = TBLK * BL  # block columns (t'*BL + b) = 256

    nc = bacc.Bacc("TRN2", target_bir_lowering=False, debug=False)
    x = nc.dram_tensor("x", [t_total, BL, F], FD, kind="ExternalInput")
    Wz = nc.dram_tensor("Wz", [F, H], FD, kind="ExternalInput")
    Wh = nc.dram_tensor("Wh", [F, H], FD, kind="ExternalInput")
    Uz = nc.dram_tensor("Uz", [H, H], FD, kind="ExternalInput")
    Uh = nc.dram_tensor("Uh", [H, H], FD, kind="ExternalInput")
    hs = nc.dram_tensor("hs", [t_total, BL, H], FD, kind="ExternalOutput")

    x_flat = x.rearrange("t b f -> (t b) f")
    hs_flat = hs.rearrange("t b h -> (t b) h")

    Sig = mybir.ActivationFunctionType.Sigmoid

    with tile.TileContext(nc) as tc:
        with (
            tc.tile_pool(name="const", bufs=1) as constp,
            tc.tile_pool(name="setup", bufs=2) as setupp,
            tc.tile_pool(name="state", bufs=1) as statep,
            tc.tile_pool(name="xblk", bufs=2) as xblkp,
            tc.tile_pool(name="work", bufs=3) as workp,
            tc.tile_pool(name="step", bufs=3) as stepp,
            tc.tile_pool(name="ps_gate", bufs=1, space="PSUM") as ps_gate,
            tc.tile_pool(name="ps_tr", bufs=2, space="PSUM") as ps_tr,
        ):
            ident = constp.tile([128, 128], FD, tag="ident")
            make_identity(nc, ident)
            ident_bf = constp.tile([128, 128], BF, tag="identbf")
            nc.vector.tensor_copy(ident_bf, ident)

            # --- U blocks, bf16: lhsT for (gate, kc, mt) ---
            Ub = {}
            for g, Usrc in (("z", Uz), ("h", Uh)):
                for kc in range(KC):
                    stage = setupp.tile(
                        [128, H], FD, tag=f"stage{g}{kc}", name=f"stage{g}{kc}"
                    )
                    nc.sync.dma_start(out=stage, in_=Usrc[kc * 128 : (kc + 1) * 128, :])
                    ub = constp.tile([128, H], BF, tag=f"U{g}{kc}")
                    nc.vector.tensor_copy(ub, stage)
                    Ub[(g, kc)] = ub

            # --- W blocks, bf16: Wcat = [Wz | Wh] along output dim ---
            Wb = []
            for fc in range(FC):
                wtile = constp.tile([128, 2 * H], BF, tag=f"W{fc}")
                for si, Wsrc in enumerate((Wz, Wh)):
                    stage = setupp.tile(
                        [128, H], FD, tag=f"stageW{fc}{si}", name=f"stageW{fc}{si}"
                    )
                    nc.sync.dma_start(out=stage, in_=Wsrc[fc * 128 : (fc + 1) * 128, :])
                    nc.vector.tensor_copy(wtile[:, si * H : (si + 1) * H], stage)
                Wb.append(wtile)

            # --- persistent state ---
            # hsT[128, col=t'*BL+b, kc] bf16: h states of the current block.
            hsT = statep.tile([128, NB, KC], BF)
            nc.vector.memset(hsT[:, (TBLK - 1) * BL :, :], 0.0)
            # fr[128, slot, {f,r}, b, kc] bf16: the split-h moving operand,
            # double-slotted by step parity.  Slot 1 = state before step 0.
            fr = statep.tile([128, 2, 2, BL, KC], BF)
            nc.vector.memset(fr[:, 1, :, :, :], 0.0)

            with tc.For_i(0, nblk, 1, staggered_reset=True) as blk:
                row0 = blk * NB

                # --- load x block and transpose: xT[fc] = x_blk.T chunk ---
                xT = [
                    xblkp.tile([128, NB], BF, tag=f"xT{fc}", name=f"xT{fc}")
                    for fc in range(FC)
                ]
                for pt in range(PT):
                    xin = workp.tile([128, F], FD, tag="xin", bufs=3)
                    nc.sync.dma_start(out=xin, in_=x_flat[ds(row0 + pt * 128, 128), :])
                    for fc in range(FC):
                        pst = ps_tr.tile([128, 128], FD, tag="tr")
                        nc.tensor.transpose(
                            pst, xin[:, fc * 128 : (fc + 1) * 128], ident
                        )
                        nc.vector.tensor_copy(xT[fc][:, pt * 128 : (pt + 1) * 128], pst)

                # --- projections into PSUM ---
                # psg_*[128, col, mt]: step slab [:, cur, :] is contiguous.
                # Full-tile memset opens the banks each block (no start=True
                # anywhere: a start matmul clears the whole PSUM bank, which
                # is unsafe with multiple groups per bank; accumulate-onto-0
                # equals overwrite for either has_written state).
                psg_z = ps_gate.tile([128, NB, KC], FD, tag="psz")
                psg_h = ps_gate.tile([128, NB, KC], FD, tag="psh")
                nc.vector.memset(psg_z, 0.0)
                nc.vector.memset(psg_h, 0.0)
                for gi, psg in ((0, psg_z), (1, psg_h)):
                    for mt in range(KC):
                        lhs_sl = slice(gi * H + mt * 128, gi * H + (mt + 1) * 128)
                        for fc in range(FC):
                            nc.tensor.matmul(
                                psg[:, :, mt],
                                lhsT=Wb[fc][:, lhs_sl],
                                rhs=xT[fc],
                                start=False,
                                stop=False,
                                skip_group_check=True,
                            )

                # --- recurrence over this block ---
                for tp in range(TBLK):
                    cur = slice(tp * BL, (tp + 1) * BL)
                    prev = (
                        slice((tp - 1) * BL, tp * BL)
                        if tp > 0
                        else slice((TBLK - 1) * BL, TBLK * BL)
                    )
                    s = tp % 2
                    sp = 1 - s  # slot holding [f|r] of step tp-1
                    # z-gate matmuls first, then h-gate: the sigmoid path
                    # (z, w, f) hides under the h-phase.
                    # The out AP aliases its 4 columns twice (zero-stride
                    # leading free dim) so the f- and r-products of the 8
                    # moving columns accumulate into the same PSUM columns.
                    for gi, g, psg in ((0, "z", psg_z), (1, "h", psg_h)):
                        for kc in range(KC):
                            for mt in range(KC):
                                out = psg[:, cur, mt]
                                out2 = out.copy()
                                out2.ap = out2.ap[:1] + [[0, 2]] + out2.ap[1:]
                                nc.tensor.matmul(
                                    out2,
                                    lhsT=Ub[(g, kc)][:, mt * 128 : (mt + 1) * 128],
                                    rhs=fr[:, sp, :, :, kc],
                                    start=False,
                                    stop=False,
                                    skip_group_check=True,
                                )
                    # h_{t-1} = f+r materialized lazily on GPSIMD (used by the
                    # output buffer and the next step's f; off critical path)
                    if tp > 0:
                        nc.gpsimd.tensor_add(
                            hsT[:, prev, :], fr[:, sp, 0, :, :], fr[:, sp, 1, :, :]
                        )
                    z = stepp.tile([128, BL, KC], FD, tag="z")
                    nc.scalar.activation(z, psg_z[:, cur, :], Sig)
                    # w = 1-z = sigmoid(-x) on ACT: serializes behind the
                    # sigmoid, but the DVE tail op reading an ACT-written
                    # tensor avoids the ~160ns DVE->DVE read-write bubble
                    # that a DVE-computed w would cost on the critical path
                    w = stepp.tile([128, BL, KC], FD, tag="w")
                    nc.scalar.activation(w, psg_z[:, cur, :], Sig, scale=-1.0)
                    # f = z * h_prev on GPSIMD, parallel to DVE
                    nc.gpsimd.tensor_mul(fr[:, s, 0, :, :], z, hsT[:, prev, :])
                    # exposed tail after the last h-gate matmul: ONE fused op
                    # r = (ps_h max 0) * w
                    nc.vector.scalar_tensor_tensor(
                        fr[:, s, 1, :, :],
                        psg_h[:, cur, :],
                        0.0,
                        w,
                        mybir.AluOpType.max,
                        mybir.AluOpType.mult,
                    )

                # last step's h for this block
                lastsl = slice((TBLK - 1) * BL, TBLK * BL)
                lsp = (TBLK - 1) % 2
                nc.gpsimd.tensor_add(
                    hsT[:, lastsl, :], fr[:, lsp, 0, :, :], fr[:, lsp, 1, :, :]
                )

                # --- transpose back to natural layout and store ---
                for ct in range(PT):
                    hnat = workp.tile([128, H], FD, tag="hnat", bufs=3)
                    for c in range(KC):
                        pst = ps_tr.tile([128, 128], BF, tag="trbf")
                        nc.tensor.transpose(
                            pst, hsT[:, ct * 128 : (ct + 1) * 128, c], ident_bf
                        )
                        nc.vector.tensor_copy(hnat[:, c * 128 : (c + 1) * 128], pst)
                    nc.sync.dma_start(out=hs_flat[ds(row0 + ct * 128, 128), :], in_=hnat)

    nc.finalize()
    return nc


def kernel(x, Wz, Wh, Uz, Uh):
    from concourse.bass_utils import run_bass_kernel_spmd

    t_total = x.shape[0]
    key = (t_total,)
    if key not in _CACHED:
        _CACHED[key] = _build_nc(t_total)
    nc = _CACHED[key]

    x = np.ascontiguousarray(np.asarray(x, dtype=np.float32))
    Wz = np.ascontiguousarray(np.asarray(Wz, dtype=np.float32))
    Wh = np.ascontiguousarray(np.asarray(Wh, dtype=np.float32))
    Uz = np.ascontiguousarray(np.asarray(Uz, dtype=np.float32))
    Uh = np.ascontiguousarray(np.asarray(Uh, dtype=np.float32))

    in_maps = []
    for c in range(NCORES):
        in_maps.append(
            {
                "x": np.ascontiguousarray(x[:, c * BL : (c + 1) * BL, :]),
                "Wz": Wz,
                "Wh": Wh,
                "Uz": Uz,
                "Uh": Uh,
            }
        )

    trace = os.environ.get("LGRU_TRACE", "0") == "1"
    res = run_bass_kernel_spmd(
        nc, in_maps, core_ids=list(range(NCORES)), trace=trace
    )
    if trace and res.exec_time_ns is not None:
        print(f"HW exec time: {res.exec_time_ns} ns")
        kernel.last_exec_time_ns = res.exec_time_ns
        kernel.last_trace = res.instructions_and_trace
    out = np.concatenate([r["hs"] for r in res.results], axis=1)
    return out
